# revision 20
# baseline (speedup 1.0000x reference)
"""Self-contained TRN2 Bass kernel for causal self-attention (B=2,T=2048,D=1024,H=16).

kernel(**inputs) takes the full unsharded inputs and returns the full output.
Sharding: 8 NeuronCores; core c -> batch b=c//4, head-group g=c%4 (4 heads).
Each core runs projections + RoPE + causal flash-style attention (transposed
scores, deferred softmax normalization) + a partial output projection; the
host sums the 4 per-batch partials and adds the output bias.

Weight columns are permuted host-side so each 128-partition projection tile
holds [hA.x1|hA.x2|hB.x1|hB.x2] (32 rows each); RoPE is then bias-fused into
the PSUM->SBUF cast plus a 32-partition swap (4 small SBUF DMAs) and 3 DVE
ops per tile, with no partition-merge copies.
"""

import math
from contextlib import ExitStack

import numpy as np

import concourse.bass as bass
import concourse.tile as tile
from concourse import bacc, mybir

F32 = mybir.dt.float32
F32R = mybir.dt.float32r
BF16 = mybir.dt.bfloat16

DEBUG_DUMPS = False

B, T, D, H, HD = 2, 2048, 1024, 16, 64
P = 128
KT = D // P            # 8 k-slabs for projections
NT = T // P            # 16 t/k tiles
QS = 512               # q-slab width for attention
NQS = T // QS          # 4 q-slabs
HPG = 4                # heads per core


def build_nc(num_devices=8):
    nc = bacc.Bacc("TRN2", target_bir_lowering=False, debug=False,
                   num_devices=num_devices)
    ext = dict(kind="ExternalInput")
    xT = nc.dram_tensor("xT", [D, T], BF16, **ext).ap()
    wq = nc.dram_tensor("wq", [D, 2 * P], BF16, **ext).ap()
    wk = nc.dram_tensor("wk", [D, 2 * P], BF16, **ext).ap()
    wv = nc.dram_tensor("wv", [D, 2 * P], BF16, **ext).ap()
    wo = nc.dram_tensor("wo", [2 * P, D], F32R, **ext).ap()
    c1 = nc.dram_tensor("c1", [P, T], BF16, **ext).ap()
    c2 = nc.dram_tensor("c2", [P, T], BF16, **ext).ap()
    bq2 = nc.dram_tensor("bq2", [P, 2], F32, **ext).ap()
    bk2 = nc.dram_tensor("bk2", [P, 2], F32, **ext).ap()
    bvr = nc.dram_tensor("bvr", [1, 2 * P], BF16, **ext).ap()
    m0 = nc.dram_tensor("m0", [P, P], BF16, **ext).ap()
    ones_in = nc.dram_tensor("ones_in", [P, P], BF16, **ext).ap()
    out = nc.dram_tensor("out", [T, D], BF16, kind="ExternalOutput").ap()
    dbg = None
    if DEBUG_DUMPS:
        dbg = {
            "dbg_y": nc.dram_tensor("dbg_y", [P, T], BF16,
                                    kind="ExternalOutput").ap(),
            "dbg_r": nc.dram_tensor("dbg_r", [NQS, HPG, QS], F32,
                                    kind="ExternalOutput").ap(),
            "dbg_rr": nc.dram_tensor("dbg_rr", [NQS, HPG, QS], F32,
                                     kind="ExternalOutput").ap(),
            "dbg_ex": nc.dram_tensor("dbg_ex", [P, 2, QS], BF16,
                                     kind="ExternalOutput").ap(),
            "dbg_avs": nc.dram_tensor("dbg_avs", [HD, HPG, QS], F32,
                                      kind="ExternalOutput").ap(),
            "dbg_at": nc.dram_tensor("dbg_at", [P, T], F32R,
                                     kind="ExternalOutput").ap(),
        }

    with tile.TileContext(nc) as tc:
        _body(tc, xT, wq, wk, wv, wo, c1, c2, bq2, bk2, bvr, m0,
              ones_in, out, dbg)
    nc.compile()
    return nc


def _body(tc, xT, wq, wk, wv, wo, c1, c2, bq2, bk2, bvr, m0,
          ones_in, out, dbg=None):
    nc = tc.nc
    Ident = mybir.ActivationFunctionType.Identity
    Exp = mybir.ActivationFunctionType.Exp

    with ExitStack() as outer:
        consts = outer.enter_context(tc.tile_pool(name="consts", bufs=1))
        wpool = outer.enter_context(tc.tile_pool(name="w", bufs=1))
        qk = outer.enter_context(tc.tile_pool(name="qk", bufs=1))
        vp = outer.enter_context(tc.tile_pool(name="v", bufs=1))
        atp = outer.enter_context(tc.tile_pool(name="at", bufs=1))

        bq_s = consts.tile([P, 2], F32, tag="bq")
        bk_s = consts.tile([P, 2], F32, tag="bk")
        bv_s = consts.tile([1, 2 * P], BF16, tag="bv")
        nc.gpsimd.dma_start(bv_s[:], bvr)
        m0_s = consts.tile([P, P], BF16, tag="m0")
        nc.gpsimd.dma_start(m0_s[:], m0)
        ones_s = consts.tile([1, P], BF16, tag="ones")
        nc.gpsimd.dma_start(ones_s[:], ones_in[0:1, :])

        v_s = vp.tile([P, NT, HPG, HD + 1], BF16)
        nc.gpsimd.dma_start(
            v_s[:, :, :, HD:HD + 1],
            ones_in[:, 0:NT * HPG].rearrange("p (t h) -> p t h", t=NT)[:, :, :, None],
        )
        at0 = atp.tile([P, T], F32R, tag="at0")
        at1 = atp.tile([P, T], F32R, tag="at1")
        at_tiles = (at0, at1)

        # ================= phase A: projections + RoPE =================
        qc, kc = [], []
        with ExitStack() as pha:
            xtp = pha.enter_context(tc.tile_pool(name="xt", bufs=1))
            cscp = pha.enter_context(tc.tile_pool(name="cs", bufs=1))
            rawp = pha.enter_context(tc.tile_pool(name="raw", bufs=4))
            rtmp = pha.enter_context(tc.tile_pool(name="rtmp", bufs=3))

            # weights first (small), then xT in per-slab chunks so the first
            # matmuls can start as soon as slab 0 lands
            wq_s = wpool.tile([P, KT, 2 * P], BF16, tag="wq")
            nc.scalar.dma_start(
                wq_s[:], wq.rearrange("(ko ki) m -> ki ko m", ki=P))
            wk_s = wpool.tile([P, KT, 2 * P], BF16, tag="wk")
            nc.scalar.dma_start(
                wk_s[:], wk.rearrange("(ko ki) m -> ki ko m", ki=P))
            nc.scalar.dma_start(bq_s[:], bq2)
            nc.scalar.dma_start(bk_s[:], bk2)
            xts = []
            xTr = xT.rearrange("(ko ki) t -> ki ko t", ki=P)
            for kt in range(KT):
                xc = xtp.tile([P, T], BF16, tag=f"xt{kt}")
                nc.sync.dma_start(xc[:], xTr[:, kt, :])
                xts.append(xc)
            c1_s = cscp.tile([P, T], BF16, tag="c1")
            nc.scalar.dma_start(c1_s[:], c1)
            c2_s = cscp.tile([P, T], BF16, tag="c2")
            nc.scalar.dma_start(c2_s[:], c2)

            # Q/K projections, one [128, T] pair-tile at a time.
            # Pair-tile m of q (or k) holds heads 2m,2m+1 as
            # [x1(32)|x2(32)|x1(32)|x2(32)] on partitions (weights are
            # permuted host-side). RoPE: y = raw*C1 + swap32(raw)*C2.
            with ExitStack() as qkps:
                ps_qk = qkps.enter_context(
                    tc.tile_pool(name="psqk", bufs=2, space="PSUM"))
                for name, w_s, b_s, m in (("q", wq_s, bq_s, 0),
                                          ("q", wq_s, bq_s, 1),
                                          ("k", wk_s, bk_s, 0),
                                          ("k", wk_s, bk_s, 1)):
                    ps = ps_qk.tile([P, T], F32, tag="ps")
                    for kt in range(KT):
                        for n in range(T // 512):
                            nc.tensor.matmul(
                                ps[:, n * 512:(n + 1) * 512],
                                w_s[:, kt, m * P:(m + 1) * P],
                                xts[kt][:, n * 512:(n + 1) * 512],
                                start=(kt == 0), stop=(kt == KT - 1),
                            )
                    raw = rawp.tile([P, T], BF16, tag="raw")
                    nc.scalar.activation(out=raw[:], in_=ps[:], func=Ident,
                                         bias=b_s[:, m:m + 1], scale=1.0)
                    swap = rawp.tile([P, T], BF16, tag="swap")
                    for j in range(2):
                        nc.gpsimd.dma_start(
                            swap[64 * j:64 * j + 32, :],
                            raw[64 * j + 32:64 * j + 64, :])
                        nc.gpsimd.dma_start(
                            swap[64 * j + 32:64 * j + 64, :],
                            raw[64 * j:64 * j + 32, :])
                    t1 = rtmp.tile([P, T], BF16, tag="rt")
                    nc.vector.tensor_mul(t1[:], raw[:], c1_s[:])
                    t2 = rtmp.tile([P, T], BF16, tag="rt")
                    nc.vector.tensor_mul(t2[:], swap[:], c2_s[:])
                    y = qk.tile([P, T], BF16, tag=f"y{name}{m}")
                    nc.vector.tensor_add(y[:], t1[:], t2[:])
                    (qc if name == "q" else kc).append(y)
                if dbg is not None:
                    nc.sync.dma_start(dbg["dbg_y"], qc[0][:])

            # V projection -> v_s [128 tokens, kt, h, 65] with ones column.
            # kt-inner per token-pair so only one PSUM bank is live per tile.
            with ExitStack() as vps:
                ps_v = vps.enter_context(
                    tc.tile_pool(name="psv", bufs=4, space="PSUM"))
                wv_s = wpool.tile([P, KT, 2 * P], BF16, tag="wv")
                nc.scalar.dma_start(
                    wv_s[:], wv.rearrange("(ko ki) m -> ki ko m", ki=P))
                for tp in range(8):
                    vt = ps_v.tile([P, 2, 2 * P], F32, tag="vps")
                    for kt in range(KT):
                        for half in range(2):
                            t = 2 * tp + half
                            nc.tensor.matmul(
                                vt[:, half, :],
                                xts[kt][:, t * P:(t + 1) * P],
                                wv_s[:, kt, :],
                                start=(kt == 0 and half == 0), stop=False,
                            )
                    for half in range(2):
                        nc.tensor.matmul(
                            vt[:, half, :], ones_s[:], bv_s[:],
                            start=False, stop=(half == 1),
                        )
                    for half in range(2):
                        t = 2 * tp + half
                        nc.vector.tensor_copy(
                            v_s[:, t, :, 0:HD],
                            vt[:, half, :].rearrange("p (h d) -> p h d", h=HPG),
                        )

        # ================= phase B: attention =================
        with ExitStack() as phb:
            expp = phb.enter_context(tc.tile_pool(name="exp", bufs=6))
            avsp = phb.enter_context(tc.tile_pool(name="avs", bufs=2))
            rp = phb.enter_context(tc.tile_pool(name="r", bufs=2))
            rrp = phb.enter_context(tc.tile_pool(name="rr", bufs=2))
            ps_sc = phb.enter_context(
                tc.tile_pool(name="pssc", bufs=2, space="PSUM"))
            ps_av = phb.enter_context(
                tc.tile_pool(name="psav", bufs=1, space="PSUM"))
            drp = phb.enter_context(
                tc.tile_pool(name="dr", bufs=2, space="DRAM"))
            wop = phb.enter_context(tc.tile_pool(name="wo", bufs=1))
            outb = phb.enter_context(tc.tile_pool(name="outb", bufs=3))

            wo_s = wop.tile([P, 2, D], F32R)
            nc.scalar.dma_start(
                wo_s[:], wo.rearrange("(ko ki) m -> ki ko m", ki=P))

            staged = {}

            def emit_normalize(qs, fine=False):
                avs, rr = staged.pop(qs)
                rrs = {h: rr[:, h, :] for h in range(HPG)}
                if not fine:
                    for tg in range(2):
                        for i in range(2):
                            h = 2 * tg + i
                            nc.vector.tensor_mul(
                                at_tiles[tg][i * HD:(i + 1) * HD,
                                             qs * QS:(qs + 1) * QS],
                                avs[0:HD, h, :], rrs[h][:],
                            )
                else:
                    # per-qt columns so outproj can chase the muls
                    for qt in range(4 * qs, 4 * qs + 4):
                        c0 = qt * P - qs * QS
                        for tg in range(2):
                            for i in range(2):
                                h = 2 * tg + i
                                nc.vector.tensor_mul(
                                    at_tiles[tg][i * HD:(i + 1) * HD,
                                                 qt * P:(qt + 1) * P],
                                    avs[0:HD, h, c0:c0 + P],
                                    rrs[h][:, c0:c0 + P],
                                )
                        emit_outproj_qt(qt)

            def emit_outproj_qt(qt):
                ps = ps_sc.tile([P, 2, QS], F32, tag="sc")
                ob = outb.tile([P, D], BF16, tag="ob")
                for nb in range(2):
                    for ktg in range(2):
                        nc.tensor.matmul(
                            ps[:, nb, :],
                            at_tiles[ktg][:, qt * P:(qt + 1) * P],
                            wo_s[:, ktg, nb * 512:(nb + 1) * 512],
                            start=(ktg == 0), stop=(ktg == 1),
                        )
                # stage psum->sbuf (bf16 partials halve the out DMA), DMA out
                nc.vector.tensor_copy(ob[:, 0:512], ps[:, 0, :])
                nc.vector.tensor_copy(ob[:, 512:1024], ps[:, 1, :])
                nc.sync.dma_start(out[qt * P:(qt + 1) * P, :], ob[:])

            def emit_outproj(qs):
                for qt in range(4 * qs, 4 * qs + 4):
                    emit_outproj_qt(qt)

            QORDER = [0, 1, 2, 3]
            for qidx, qs in enumerate(QORDER):
                av = ps_av.tile([HD + 1, HPG, QS], F32, tag="av")
                n_kt = 4 * qs + 4
                for kt in range(n_kt):
                    qoff = max(0, kt * P - qs * QS)
                    q0 = qs * QS + qoff
                    qext = QS - qoff
                    diag = kt * P >= qs * QS
                    for pair in range(2):
                        sc = ps_sc.tile([P, 2, QS], F32, tag="sc")
                        for i in range(2):
                            hs = slice(64 * i, 64 * i + 64)
                            nc.tensor.matmul(
                                sc[:, i, qoff:QS],
                                kc[pair][hs, kt * P:(kt + 1) * P],
                                qc[pair][hs, q0:q0 + qext],
                                start=True, stop=True,
                                tile_position=(64 * i, 0),
                            )
                        ex = expp.tile([P, 2, QS], BF16, tag="ex")
                        nc.scalar.activation(
                            out=ex[:, :, qoff:QS], in_=sc[:, :, qoff:QS],
                            func=Exp, scale=1.0,
                        )
                        if dbg is not None and qs == 3 and kt == 10 and pair == 0:
                            nc.sync.dma_start(dbg["dbg_ex"], ex[:])
                        if diag:
                            # diagonal k-tile: zero strictly-upper corner (k > q)
                            nc.gpsimd.tensor_mul(
                                ex[:, :, qoff:qoff + P],
                                ex[:, :, qoff:qoff + P],
                                m0_s[:, None, :].broadcast_to([P, 2, P]),
                            )
                        for i in range(2):
                            h = 2 * pair + i
                            nc.tensor.matmul(
                                av[:, h, qoff:QS],
                                v_s[:, kt, h, :],
                                ex[:, i, qoff:QS],
                                start=(kt == 0), stop=(kt == n_kt - 1),
                            )
                # softmax denominators -> r = 1/sums on DVE
                r_row = rp.tile([1, HPG, QS], F32, tag="r")
                nc.vector.reciprocal(r_row[:], av[HD:HD + 1, :, :])
                d_r = drp.tile([HPG * QS], F32, tag="dr")
                nc.sync.dma_start(d_r[None, :],
                                  r_row.rearrange("o h q -> o (h q)"))
                rr = rrp.tile([HD, HPG, QS], F32, tag="rr")
                nc.sync.dma_start(
                    rr[:], d_r[None, :].broadcast_to([HD, HPG * QS])
                    .rearrange("p (h q) -> p h q", h=HPG))
                # stage AV psum to SBUF (frees psum fast)
                avs = avsp.tile([HD, HPG, QS], F32, tag="avs")
                nc.vector.tensor_copy(avs[:], av[0:HD, :, :])
                staged[qs] = (avs, rr)
                if dbg is not None:
                    nc.sync.dma_start(dbg["dbg_r"][qs], r_row[0])
                    nc.sync.dma_start(dbg["dbg_rr"][qs], rr[0])
                    if qs == 3:
                        nc.sync.dma_start(dbg["dbg_avs"], avs[:])

                if qidx > 0:
                    emit_normalize(QORDER[qidx - 1])
                    emit_outproj(QORDER[qidx - 1])

            emit_normalize(QORDER[-1], fine=True)
            if dbg is not None:
                nc.sync.dma_start(dbg["dbg_at"], at0[:])


# ---------------- host-side prep ----------------

def _perm(g):
    # pair-tile layout: tile m holds heads (2m, 2m+1); partition
    # 64*j + 32*s + c  ->  column 256*g + 64*(2m+j) + 2*c + s
    perm = []
    for m in range(2):
        for j in range(2):
            for s in range(2):
                for c in range(32):
                    perm.append(256 * g + 64 * (2 * m + j) + 2 * c + s)
    return np.array(perm)


def host_inputs(inputs, c):
    b, g = c // 4, c % 4
    x, cos, sin = inputs["x"], inputs["cos"], inputs["sin"]
    Wq, bq, Wk, bk = inputs["Wq"], inputs["bq"], inputs["Wk"], inputs["bk"]
    Wv, bv, Wo = inputs["Wv"], inputs["bv"], inputs["Wo"]
    perm = _perm(g)
    s = math.sqrt(1.0 / math.sqrt(HD))
    cosT = np.ascontiguousarray(cos[0, 0].T) * s    # [32, T]
    sinT = np.ascontiguousarray(sin[0, 0].T) * s
    # C1: cos pattern on every 32-row block; C2: [-sin; +sin] per 64-block
    c1 = np.tile(cosT, (4, 1))
    c2 = np.concatenate([-sinT, sinT, -sinT, sinT], axis=0)
    f32 = np.float32
    import ml_dtypes
    bf16 = ml_dtypes.bfloat16
    return {
        "xT": np.ascontiguousarray(x[b].T).astype(bf16),
        "wq": np.ascontiguousarray(Wq[perm, :].T).astype(bf16),
        "wk": np.ascontiguousarray(Wk[perm, :].T).astype(bf16),
        "wv": np.ascontiguousarray(Wv[256 * g:256 * (g + 1), :].T).astype(bf16),
        "wo": np.ascontiguousarray(Wo[:, 256 * g:256 * (g + 1)].T).astype(f32),
        "c1": np.ascontiguousarray(c1).astype(bf16),
        "c2": np.ascontiguousarray(c2).astype(bf16),
        "bq2": np.ascontiguousarray(bq[perm].reshape(2, P).T).astype(f32),
        "bk2": np.ascontiguousarray(bk[perm].reshape(2, P).T).astype(f32),
        "bvr": np.ascontiguousarray(
            bv[256 * g:256 * (g + 1)].reshape(1, 2 * P)).astype(bf16),
        "m0": np.ascontiguousarray(
            (np.arange(P)[None, :] >= np.arange(P)[:, None])).astype(bf16),
        "ones_in": np.ones((P, P), bf16),
    }


def host_gather(results, bo):
    out = np.zeros((B, T, D), np.float32)
    for c in range(8):
        out[c // 4] += results[c]["out"].astype(np.float32)
    out += bo[None, None, :]
    return out


_NC_CACHE = {}


def _get_nc():
    if "nc" not in _NC_CACHE:
        _NC_CACHE["nc"] = build_nc(num_devices=8)
    return _NC_CACHE["nc"]


def kernel(**inputs):
    inputs = {k: np.asarray(v) for k, v in inputs.items()}
    nc = _get_nc()
    from concourse.bass_utils import run_bass_kernel_spmd
    in_maps = [host_inputs(inputs, c) for c in range(8)]
    res = run_bass_kernel_spmd(nc, in_maps, core_ids=list(range(8)))
    return host_gather(res.results, inputs["bo"].astype(np.float32))


# revision 24
# speedup vs baseline: 1.1197x; 1.1197x over previous
"""Self-contained TRN2 Bass kernel for causal self-attention (B=2,T=2048,D=1024,H=16).

kernel(**inputs) takes the full unsharded inputs and returns the full output.
Sharding: 8 NeuronCores; core c -> batch b=c//4, head-group g=c%4 (4 heads).
Each core runs projections + RoPE + causal flash-style attention (transposed
scores, deferred softmax normalization) + a partial output projection; the
host sums the 4 per-batch partials and adds the output bias.

Weight columns are permuted host-side so each 128-partition projection tile
holds [hA.x1|hA.x2|hB.x1|hB.x2] (32 rows each); RoPE is then bias-fused into
the PSUM->SBUF cast plus a 32-partition swap (4 small SBUF DMAs) and 3 DVE
ops per tile, with no partition-merge copies.
"""

import math
from contextlib import ExitStack

import numpy as np

import concourse.bass as bass
import concourse.tile as tile
from concourse import bacc, mybir

F32 = mybir.dt.float32
F32R = mybir.dt.float32r
BF16 = mybir.dt.bfloat16

DEBUG_DUMPS = False

B, T, D, H, HD = 2, 2048, 1024, 16, 64
P = 128
KT = D // P            # 8 k-slabs for projections
NT = T // P            # 16 t/k tiles
QS = 512               # q-slab width for attention
NQS = T // QS          # 4 q-slabs
HPG = 4                # heads per core


def build_nc(num_devices=8):
    nc = bacc.Bacc("TRN2", target_bir_lowering=False, debug=False,
                   num_devices=num_devices)
    ext = dict(kind="ExternalInput")
    xT = nc.dram_tensor("xT", [D, T], BF16, **ext).ap()
    wq = nc.dram_tensor("wq", [D, 2 * P], BF16, **ext).ap()
    wk = nc.dram_tensor("wk", [D, 2 * P], BF16, **ext).ap()
    wv = nc.dram_tensor("wv", [D, 2 * P], BF16, **ext).ap()
    wo = nc.dram_tensor("wo", [2 * P, D], F32R, **ext).ap()
    c1 = nc.dram_tensor("c1", [P, T], BF16, **ext).ap()
    c2 = nc.dram_tensor("c2", [P, T], BF16, **ext).ap()
    bq2 = nc.dram_tensor("bq2", [P, 2], F32, **ext).ap()
    bk2 = nc.dram_tensor("bk2", [P, 2], F32, **ext).ap()
    bvr = nc.dram_tensor("bvr", [1, 2 * P], BF16, **ext).ap()
    m0 = nc.dram_tensor("m0", [P, P], BF16, **ext).ap()
    ones_in = nc.dram_tensor("ones_in", [P, P], BF16, **ext).ap()
    out = nc.dram_tensor("out", [T, D], BF16, kind="ExternalOutput").ap()
    dbg = None
    if DEBUG_DUMPS:
        dbg = {
            "dbg_y": nc.dram_tensor("dbg_y", [P, T], BF16,
                                    kind="ExternalOutput").ap(),
            "dbg_r": nc.dram_tensor("dbg_r", [NQS, HPG, QS], F32,
                                    kind="ExternalOutput").ap(),
            "dbg_rr": nc.dram_tensor("dbg_rr", [NQS, HPG, QS], F32,
                                     kind="ExternalOutput").ap(),
            "dbg_ex": nc.dram_tensor("dbg_ex", [P, 2, QS], BF16,
                                     kind="ExternalOutput").ap(),
            "dbg_avs": nc.dram_tensor("dbg_avs", [HD, HPG, QS], F32,
                                      kind="ExternalOutput").ap(),
            "dbg_at": nc.dram_tensor("dbg_at", [P, T], F32R,
                                     kind="ExternalOutput").ap(),
        }

    with tile.TileContext(nc) as tc:
        _body(tc, xT, wq, wk, wv, wo, c1, c2, bq2, bk2, bvr, m0,
              ones_in, out, dbg)
    nc.compile()
    return nc


def _body(tc, xT, wq, wk, wv, wo, c1, c2, bq2, bk2, bvr, m0,
          ones_in, out, dbg=None):
    nc = tc.nc
    Ident = mybir.ActivationFunctionType.Identity
    Exp = mybir.ActivationFunctionType.Exp
    Ln = mybir.ActivationFunctionType.Ln

    with ExitStack() as outer:
        consts = outer.enter_context(tc.tile_pool(name="consts", bufs=1))
        wpool = outer.enter_context(tc.tile_pool(name="w", bufs=1))
        qk = outer.enter_context(tc.tile_pool(name="qk", bufs=1))
        vp = outer.enter_context(tc.tile_pool(name="v", bufs=1))
        atp = outer.enter_context(tc.tile_pool(name="at", bufs=1))

        bq_s = consts.tile([P, 2], F32, tag="bq")
        bk_s = consts.tile([P, 2], F32, tag="bk")
        bv_s = consts.tile([1, 2 * P], BF16, tag="bv")
        nc.gpsimd.dma_start(bv_s[:], bvr)
        m0_s = consts.tile([P, P], BF16, tag="m0")
        nc.gpsimd.dma_start(m0_s[:], m0)
        ones_s = consts.tile([1, P], BF16, tag="ones")
        nc.gpsimd.dma_start(ones_s[:], ones_in[0:1, :])

        v_s = vp.tile([P, NT, HPG, HD + 1], BF16)
        nc.gpsimd.dma_start(
            v_s[:, :, :, HD:HD + 1],
            ones_in[:, 0:NT * HPG].rearrange("p (t h) -> p t h", t=NT)[:, :, :, None],
        )
        at0 = atp.tile([P, T], F32R, tag="at0")
        at1 = atp.tile([P, T], F32R, tag="at1")
        at_tiles = (at0, at1)

        # ================= phase A: projections + RoPE =================
        qc, kc = [], []
        with ExitStack() as pha:
            xtp = pha.enter_context(tc.tile_pool(name="xt", bufs=1))
            cscp = pha.enter_context(tc.tile_pool(name="cs", bufs=1))
            rawp = pha.enter_context(tc.tile_pool(name="raw", bufs=4))
            rtmp = pha.enter_context(tc.tile_pool(name="rtmp", bufs=3))

            # weights first (small), then xT in per-slab chunks so the first
            # matmuls can start as soon as slab 0 lands
            wq_s = wpool.tile([P, KT, 2 * P], BF16, tag="wq")
            nc.scalar.dma_start(
                wq_s[:], wq.rearrange("(ko ki) m -> ki ko m", ki=P))
            wk_s = wpool.tile([P, KT, 2 * P], BF16, tag="wk")
            nc.scalar.dma_start(
                wk_s[:], wk.rearrange("(ko ki) m -> ki ko m", ki=P))
            nc.scalar.dma_start(bq_s[:], bq2)
            nc.scalar.dma_start(bk_s[:], bk2)
            xts = []
            xTr = xT.rearrange("(ko ki) t -> ki ko t", ki=P)
            for kt in range(KT):
                xc = xtp.tile([P, T], BF16, tag=f"xt{kt}")
                eng = nc.sync if kt % 2 == 0 else nc.gpsimd
                eng.dma_start(xc[:], xTr[:, kt, :])
                xts.append(xc)
            c1_s = cscp.tile([P, T], BF16, tag="c1")
            nc.scalar.dma_start(c1_s[:], c1)
            c2_s = cscp.tile([P, T], BF16, tag="c2")
            nc.scalar.dma_start(c2_s[:], c2)

            # Q/K projections, one [128, T] pair-tile at a time.
            # Pair-tile m of q (or k) holds heads 2m,2m+1 as
            # [x1(32)|x2(32)|x1(32)|x2(32)] on partitions (weights are
            # permuted host-side). RoPE: y = raw*C1 + swap32(raw)*C2.
            with ExitStack() as qkps:
                ps_qk = qkps.enter_context(
                    tc.tile_pool(name="psqk", bufs=2, space="PSUM"))
                for name, w_s, b_s, m in (("q", wq_s, bq_s, 0),
                                          ("q", wq_s, bq_s, 1),
                                          ("k", wk_s, bk_s, 0),
                                          ("k", wk_s, bk_s, 1)):
                    ps = ps_qk.tile([P, T], F32, tag="ps")
                    for kt in range(KT):
                        for n in range(T // 512):
                            nc.tensor.matmul(
                                ps[:, n * 512:(n + 1) * 512],
                                w_s[:, kt, m * P:(m + 1) * P],
                                xts[kt][:, n * 512:(n + 1) * 512],
                                start=(kt == 0), stop=(kt == KT - 1),
                            )
                    raw = rawp.tile([P, T], BF16, tag="raw")
                    nc.scalar.activation(out=raw[:], in_=ps[:], func=Ident,
                                         bias=b_s[:, m:m + 1], scale=1.0)
                    swap = rawp.tile([P, T], BF16, tag="swap")
                    for j in range(2):
                        nc.gpsimd.dma_start(
                            swap[64 * j:64 * j + 32, :],
                            raw[64 * j + 32:64 * j + 64, :])
                        nc.gpsimd.dma_start(
                            swap[64 * j + 32:64 * j + 64, :],
                            raw[64 * j:64 * j + 32, :])
                    t1 = rtmp.tile([P, T], BF16, tag="rt")
                    nc.vector.tensor_mul(t1[:], raw[:], c1_s[:])
                    t2 = rtmp.tile([P, T], BF16, tag="rt")
                    nc.vector.tensor_mul(t2[:], swap[:], c2_s[:])
                    y = qk.tile([P, T], BF16, tag=f"y{name}{m}")
                    nc.vector.tensor_add(y[:], t1[:], t2[:])
                    (qc if name == "q" else kc).append(y)
                if dbg is not None:
                    nc.sync.dma_start(dbg["dbg_y"], qc[0][:])

            # V projection -> v_s [128 tokens, kt, h, 65] with ones column.
            # kt-inner per token-pair so only one PSUM bank is live per tile.
            with ExitStack() as vps:
                ps_v = vps.enter_context(
                    tc.tile_pool(name="psv", bufs=4, space="PSUM"))
                wv_s = wpool.tile([P, KT, 2 * P], BF16, tag="wv")
                nc.scalar.dma_start(
                    wv_s[:], wv.rearrange("(ko ki) m -> ki ko m", ki=P))
                for tp in range(8):
                    vt = ps_v.tile([P, 2, 2 * P], F32, tag="vps")
                    for kt in range(KT):
                        for half in range(2):
                            t = 2 * tp + half
                            nc.tensor.matmul(
                                vt[:, half, :],
                                xts[kt][:, t * P:(t + 1) * P],
                                wv_s[:, kt, :],
                                start=(kt == 0 and half == 0), stop=False,
                            )
                    for half in range(2):
                        nc.tensor.matmul(
                            vt[:, half, :], ones_s[:], bv_s[:],
                            start=False, stop=(half == 1),
                        )
                    for half in range(2):
                        t = 2 * tp + half
                        nc.vector.tensor_copy(
                            v_s[:, t, :, 0:HD],
                            vt[:, half, :].rearrange("p (h d) -> p h d", h=HPG),
                        )

        # ================= phase B: attention =================
        with ExitStack() as phb:
            expp = phb.enter_context(tc.tile_pool(name="exp", bufs=6))
            avsp = phb.enter_context(tc.tile_pool(name="avs", bufs=2))
            rp = phb.enter_context(tc.tile_pool(name="r", bufs=2))
            rrp = phb.enter_context(tc.tile_pool(name="rr", bufs=2))
            ps_sc = phb.enter_context(
                tc.tile_pool(name="pssc", bufs=2, space="PSUM"))
            ps_av = phb.enter_context(
                tc.tile_pool(name="psav", bufs=1, space="PSUM"))
            drp = phb.enter_context(
                tc.tile_pool(name="dr", bufs=2, space="DRAM"))
            wop = phb.enter_context(tc.tile_pool(name="wo", bufs=1))
            outb = phb.enter_context(tc.tile_pool(name="outb", bufs=3))

            wo_s = wop.tile([P, 2, D], F32R)
            nc.scalar.dma_start(
                wo_s[:], wo.rearrange("(ko ki) m -> ki ko m", ki=P))

            staged = {}

            def emit_normalize(qs, fine=False):
                avs, rr = staged.pop(qs)
                rrs = {h: rr[:, h, :] for h in range(HPG)}
                if not fine:
                    for tg in range(2):
                        for i in range(2):
                            h = 2 * tg + i
                            nc.vector.tensor_mul(
                                at_tiles[tg][i * HD:(i + 1) * HD,
                                             qs * QS:(qs + 1) * QS],
                                avs[0:HD, h, :], rrs[h][:],
                            )
                else:
                    # per-qt columns so outproj can chase the muls
                    for qt in range(4 * qs, 4 * qs + 4):
                        c0 = qt * P - qs * QS
                        for tg in range(2):
                            for i in range(2):
                                h = 2 * tg + i
                                nc.vector.tensor_mul(
                                    at_tiles[tg][i * HD:(i + 1) * HD,
                                                 qt * P:(qt + 1) * P],
                                    avs[0:HD, h, c0:c0 + P],
                                    rrs[h][:, c0:c0 + P],
                                )
                        emit_outproj_qt(qt)

            def emit_outproj_qt(qt):
                ps = ps_sc.tile([P, 2, QS], F32, tag="sc")
                ob = outb.tile([P, D], BF16, tag="ob")
                for nb in range(2):
                    for ktg in range(2):
                        nc.tensor.matmul(
                            ps[:, nb, :],
                            at_tiles[ktg][:, qt * P:(qt + 1) * P],
                            wo_s[:, ktg, nb * 512:(nb + 1) * 512],
                            start=(ktg == 0), stop=(ktg == 1),
                        )
                # stage psum->sbuf (bf16 partials halve the out DMA), DMA out
                nc.vector.tensor_copy(ob[:, 0:512], ps[:, 0, :])
                nc.vector.tensor_copy(ob[:, 512:1024], ps[:, 1, :])
                nc.sync.dma_start(out[qt * P:(qt + 1) * P, :], ob[:])

            def emit_outproj(qs):
                for qt in range(4 * qs, 4 * qs + 4):
                    emit_outproj_qt(qt)

            QORDER = [0, 1, 2, 3]
            for qidx, qs in enumerate(QORDER):
                av = ps_av.tile([HD + 1, HPG, QS], F32, tag="av")
                n_kt = 4 * qs + 4
                for kt in range(n_kt):
                    qoff = max(0, kt * P - qs * QS)
                    q0 = qs * QS + qoff
                    qext = QS - qoff
                    diag = kt * P >= qs * QS
                    for pair in range(2):
                        sc = ps_sc.tile([P, 2, QS], F32, tag="sc")
                        for i in range(2):
                            hs = slice(64 * i, 64 * i + 64)
                            nc.tensor.matmul(
                                sc[:, i, qoff:QS],
                                kc[pair][hs, kt * P:(kt + 1) * P],
                                qc[pair][hs, q0:q0 + qext],
                                start=True, stop=True,
                                tile_position=(64 * i, 0),
                            )
                        ex = expp.tile([P, 2, QS], BF16, tag="ex")
                        nc.scalar.activation(
                            out=ex[:, :, qoff:QS], in_=sc[:, :, qoff:QS],
                            func=Exp, scale=1.0,
                        )
                        if dbg is not None and qs == 3 and kt == 10 and pair == 0:
                            nc.sync.dma_start(dbg["dbg_ex"], ex[:])
                        if diag:
                            # diagonal k-tile: zero strictly-upper corner (k > q)
                            nc.gpsimd.tensor_mul(
                                ex[:, :, qoff:qoff + P],
                                ex[:, :, qoff:qoff + P],
                                m0_s[:, None, :].broadcast_to([P, 2, P]),
                            )
                        for i in range(2):
                            h = 2 * pair + i
                            nc.tensor.matmul(
                                av[:, h, qoff:QS],
                                v_s[:, kt, h, :],
                                ex[:, i, qoff:QS],
                                start=(kt == 0), stop=(kt == n_kt - 1),
                            )
                # stage AV psum to SBUF first (frees psum for the next slab)
                avs = avsp.tile([HD, HPG, QS], F32, tag="avs")
                nc.vector.tensor_copy(avs[:], av[0:HD, :, :])
                # r = exp(-ln(sums)) on ACT (ln/exp/identity share one table)
                ln_row = rp.tile([1, HPG, QS], F32, tag="ln")
                nc.scalar.activation(out=ln_row[:], in_=av[HD:HD + 1, :, :],
                                     func=Ln, scale=1.0)
                r_row = rp.tile([1, HPG, QS], F32, tag="r")
                nc.scalar.activation(out=r_row[:], in_=ln_row[:],
                                     func=Exp, scale=-1.0)
                d_r = drp.tile([HPG * QS], F32, tag="dr")
                nc.sync.dma_start(d_r[None, :],
                                  r_row.rearrange("o h q -> o (h q)"))
                rr = rrp.tile([HD, HPG, QS], F32, tag="rr")
                nc.sync.dma_start(
                    rr[:], d_r[None, :].broadcast_to([HD, HPG * QS])
                    .rearrange("p (h q) -> p h q", h=HPG))
                staged[qs] = (avs, rr)
                if dbg is not None:
                    nc.sync.dma_start(dbg["dbg_r"][qs], r_row[0])
                    nc.sync.dma_start(dbg["dbg_rr"][qs], rr[0])
                    if qs == 3:
                        nc.sync.dma_start(dbg["dbg_avs"], avs[:])

                if qidx > 0:
                    emit_normalize(QORDER[qidx - 1])
                    emit_outproj(QORDER[qidx - 1])

            emit_normalize(QORDER[-1], fine=True)
            if dbg is not None:
                nc.sync.dma_start(dbg["dbg_at"], at0[:])


# ---------------- host-side prep ----------------

def _perm(g):
    # pair-tile layout: tile m holds heads (2m, 2m+1); partition
    # 64*j + 32*s + c  ->  column 256*g + 64*(2m+j) + 2*c + s
    perm = []
    for m in range(2):
        for j in range(2):
            for s in range(2):
                for c in range(32):
                    perm.append(256 * g + 64 * (2 * m + j) + 2 * c + s)
    return np.array(perm)


def host_inputs(inputs, c):
    b, g = c // 4, c % 4
    x, cos, sin = inputs["x"], inputs["cos"], inputs["sin"]
    Wq, bq, Wk, bk = inputs["Wq"], inputs["bq"], inputs["Wk"], inputs["bk"]
    Wv, bv, Wo = inputs["Wv"], inputs["bv"], inputs["Wo"]
    perm = _perm(g)
    s = math.sqrt(1.0 / math.sqrt(HD))
    cosT = np.ascontiguousarray(cos[0, 0].T) * s    # [32, T]
    sinT = np.ascontiguousarray(sin[0, 0].T) * s
    # C1: cos pattern on every 32-row block; C2: [-sin; +sin] per 64-block
    c1 = np.tile(cosT, (4, 1))
    c2 = np.concatenate([-sinT, sinT, -sinT, sinT], axis=0)
    f32 = np.float32
    import ml_dtypes
    bf16 = ml_dtypes.bfloat16
    return {
        "xT": np.ascontiguousarray(x[b].T).astype(bf16),
        "wq": np.ascontiguousarray(Wq[perm, :].T).astype(bf16),
        "wk": np.ascontiguousarray(Wk[perm, :].T).astype(bf16),
        "wv": np.ascontiguousarray(Wv[256 * g:256 * (g + 1), :].T).astype(bf16),
        "wo": np.ascontiguousarray(Wo[:, 256 * g:256 * (g + 1)].T).astype(f32),
        "c1": np.ascontiguousarray(c1).astype(bf16),
        "c2": np.ascontiguousarray(c2).astype(bf16),
        "bq2": np.ascontiguousarray(bq[perm].reshape(2, P).T).astype(f32),
        "bk2": np.ascontiguousarray(bk[perm].reshape(2, P).T).astype(f32),
        "bvr": np.ascontiguousarray(
            bv[256 * g:256 * (g + 1)].reshape(1, 2 * P)).astype(bf16),
        "m0": np.ascontiguousarray(
            (np.arange(P)[None, :] >= np.arange(P)[:, None])).astype(bf16),
        "ones_in": np.ones((P, P), bf16),
    }


def host_gather(results, bo):
    out = np.zeros((B, T, D), np.float32)
    for c in range(8):
        out[c // 4] += results[c]["out"].astype(np.float32)
    out += bo[None, None, :]
    return out


_NC_CACHE = {}


def _get_nc():
    if "nc" not in _NC_CACHE:
        _NC_CACHE["nc"] = build_nc(num_devices=8)
    return _NC_CACHE["nc"]


def kernel(**inputs):
    inputs = {k: np.asarray(v) for k, v in inputs.items()}
    nc = _get_nc()
    from concourse.bass_utils import run_bass_kernel_spmd
    in_maps = [host_inputs(inputs, c) for c in range(8)]
    res = run_bass_kernel_spmd(nc, in_maps, core_ids=list(range(8)))
    return host_gather(res.results, inputs["bo"].astype(np.float32))
